# revision 45
# baseline (speedup 1.0000x reference)
"""ConsRec segment-reduce kernel for Trainium2 (8 NeuronCores, SPMD).

Strategy (v4): 16384 queries hit ~15.1K unique groups (only ~8% reuse), so
group means are never materialized.  Queries are dealt into 128-query
windows (16 per core x 8 cores) balanced by member count; each window's
member edges are duplicated per query.  Per core the device:
  1. gathers all member user-embedding rows with per-(window,chunk)
     `dma_gather` (InstDMAGatherAnt) instructions spread round-robin over
     the 4 SWDGE queues.  Descriptor generation for a gather runs on the
     Q7 cpu pair selected by queue_num (~8.6 ns/row per pair, HW
     measured), so 4 queues generate descriptors 4x in parallel -- this
     was the single-queue v3 kernel's bottleneck (~590 us/core).
     dma_gather uses int16 indices, so the bf16 user table is padded to
     128 cols and compacted per core (~63K used rows), split into 2
     chunks of <=32768 rows.  Blocks are window-major so compute streams
     window-by-window behind the gathers.
  2. builds a per-128-edge-tile one-hot (DVE is_equal vs an iota row)
     mapping edges to query columns,
  3. PE-matmuls one-hot^T @ rows into a per-window PSUM [128 queries, 64]
     accumulating each query's member-sum directly,
  4. scales by 1/count, multiplies by gathered item rows, runs the
     64->8->1 MLP + sigmoid per window.
No group-means round trip, no phase barrier, no second gather pass.
"""
import sys
sys.path.insert(0, '/opt/trn_rl_repo')
import numpy as np

import concourse.bacc as bacc
import concourse.bass as bass
import concourse.mybir as mybir
from concourse.tile import TileContext
from concourse.masks import make_identity
from concourse import library_config

N_CORES = 8
D = 64          # embedding dim
DP = 128        # padded row length (256B in bf16, dma_gather elem_size)
P = 128         # partitions / tile size / queries per window
NW = 16         # query windows per core (128 queries each)
CH = 32768      # user-table chunk rows (int16 index limit)
GT = 64         # tiles per dma_gather (8192 rows; needs single_packet=False)

F32 = mybir.dt.float32
I32 = mybir.dt.int32
I16 = mybir.dt.int16
BF16 = mybir.dt.bfloat16
BF16_NP = mybir.dt.np(BF16)


def plan(member_users, member_groups, group_inputs, item_inputs, num_groups,
         split_below=2):
    G = int(num_groups)
    B = group_inputs.shape[0]
    NB = N_CORES * NW
    QPB = B // NB
    assert QPB == P, f"expected {P} queries/bin, got {QPB}"

    if not np.all(member_groups[:-1] <= member_groups[1:]):
        order = np.argsort(member_groups, kind='stable')
        member_groups = member_groups[order]
        member_users = member_users[order]
    starts = np.searchsorted(member_groups, np.arange(G + 1))
    cnt = (starts[1:] - starts[:-1]).astype(np.int64)
    qcnt = cnt[group_inputs]

    # deal queries into bins, snake order by member count -> equal bin loads
    qorder = np.argsort(-qcnt, kind='stable')
    bin_of = np.empty(B, np.int64)
    ar = np.arange(NB)
    for r in range(QPB):
        sl = qorder[r * NB:(r + 1) * NB]
        bin_of[sl] = ar if (r % 2 == 0) else ar[::-1]
    binload = np.bincount(bin_of, weights=qcnt.astype(np.float64),
                          minlength=NB).astype(np.int64)
    bsort = np.argsort(-binload, kind='stable')
    core_bins = [[] for _ in range(N_CORES)]
    for r in range(NW):
        row = bsort[r * N_CORES:(r + 1) * N_CORES]
        if r % 2 == 1:
            row = row[::-1]
        for c in range(N_CORES):
            core_bins[c].append(row[c])
    for c in range(N_CORES):
        core_bins[c].sort(key=lambda b: -binload[b])
    bin_queries = [np.where(bin_of == b)[0] for b in range(NB)]

    # per (core, window): raw edge lists (original user ids) + query info
    edges_u = [[None] * NW for _ in range(N_CORES)]   # user ids per edge
    edges_s = [[None] * NW for _ in range(N_CORES)]   # query col per edge
    civq = np.zeros((N_CORES, P, NW), np.float32)
    qpos = np.zeros((N_CORES, P, NW), np.int64)
    qitem = np.zeros((N_CORES, P, NW), np.int64)
    for c in range(N_CORES):
        for w in range(NW):
            qs = bin_queries[core_bins[c][w]]
            gl = group_inputs[qs]
            L = cnt[gl]
            off = starts[gl]
            tot = int(L.sum())
            if tot > 0:
                csum = np.cumsum(L) - L
                pos = (np.arange(tot) - np.repeat(csum, L) + np.repeat(off, L))
                edges_u[c][w] = member_users[pos]
                edges_s[c][w] = np.repeat(np.arange(P), L).astype(np.float32)
            else:
                edges_u[c][w] = np.empty(0, np.int64)
                edges_s[c][w] = np.empty(0, np.float32)
            civq[c, :, w] = 1.0 / np.maximum(cnt[gl], 1).astype(np.float32)
            qpos[c, :, w] = qs
            qitem[c, :, w] = item_inputs[qs]

    # per-core user compaction (keeps chunk count at 2 for int16 indices)
    used_u = []
    comp = [[None] * NW for _ in range(N_CORES)]
    for c in range(N_CORES):
        allu = np.concatenate(edges_u[c]) if edges_u[c] else np.zeros(1, np.int64)
        uu = np.unique(allu)
        if len(uu) == 0:
            uu = np.zeros(1, np.int64)
        used_u.append(uu)
        for w in range(NW):
            comp[c][w] = np.searchsorted(uu, edges_u[c][w])
    U_max = max(len(u) for u in used_u)
    n_chunks = (U_max + CH - 1) // CH
    used_i, ii_c = np.unique(item_inputs, return_inverse=True)
    ii_c = ii_c.reshape(-1)

    # (window, chunk) segment tile counts, shared across cores
    T_seg = np.zeros((NW, n_chunks), np.int64)
    for c in range(N_CORES):
        for w in range(NW):
            ch = comp[c][w] // CH
            for c2 in range(n_chunks):
                n = int((ch == c2).sum())
                T_seg[w, c2] = max(T_seg[w, c2], (n + P - 1) // P)

    # stream layout: window-major; seg_off[w][c2] = first stream-tile index
    seg_off = np.zeros((NW, n_chunks), np.int64)
    off = 0
    for w in range(NW):
        for c2 in range(n_chunks):
            seg_off[w, c2] = off
            off += T_seg[w, c2]
    T_total = int(off)

    # gather blocks: one per (window, chunk), window-major so compute can
    # stream window-by-window while later windows' rows are still fetching.
    # First two windows' blocks are split in half for a faster ramp-in.
    # Blocks are spread over the 4 SWDGE queues (greedy balance) at build.
    blocks = []                           # (c2, t0, t1)
    blocks_w = []                         # window of each block
    for w in range(NW):
        # chunk emission order alternates every 2 windows so the strict
        # q=i%4 rotation (required by SWDGE sem-lane grouping) sees the
        # big-chunk/small-chunk blocks evenly across queues
        order = range(n_chunks) if (w // 2) % 2 == 0 else \
            reversed(range(n_chunks))
        for c2 in order:
            nt = int(T_seg[w, c2])
            if nt == 0:
                continue
            t0 = int(seg_off[w, c2])
            if w < split_below and nt > 1:
                h = nt // 2
                blocks.append((c2, t0, t0 + h))
                blocks.append((c2, t0 + h, t0 + nt))
                blocks_w += [w, w]
            else:
                blocks.append((c2, t0, t0 + nt))
                blocks_w.append(w)

    # per-core stream data: local int16 idx + slot per stream tile
    # (slot values 0..127 / -1 sentinel: exactly representable in bf16)
    sidx = np.zeros((N_CORES, T_total, P), np.int16)
    slot = np.full((N_CORES, T_total, P), -1.0, np.float32)
    nreal = np.zeros((N_CORES, NW, n_chunks), np.int64)
    for c in range(N_CORES):
        for w in range(NW):
            cidx = comp[c][w]
            ch = cidx // CH
            for c2 in range(n_chunks):
                m = ch == c2
                li = (cidx[m] - c2 * CH).astype(np.int16)
                sl = edges_s[c][w][m]
                ord_ = np.argsort(li, kind='stable')   # HBM locality for the
                li = li[ord_]                          # 16-row packed descs
                sl = sl[ord_]
                n = len(li)
                t0 = seg_off[w, c2]
                nt = int(T_seg[w, c2])
                if nt == 0:
                    continue
                nreal[c, w, c2] = n
                buf_i = np.zeros(nt * P, np.int16)
                buf_s = np.full(nt * P, -1.0, np.float32)
                buf_i[:n] = li
                buf_s[:n] = sl
                sidx[c, t0:t0 + nt] = buf_i.reshape(nt, P)
                slot[c, t0:t0 + nt] = buf_s.reshape(nt, P)

    # wrapped int16 index stream per gather block: [128, T_total*8]
    idx16 = np.zeros((N_CORES, P, T_total * 8), np.int16)
    blk_off = []                          # idx16 col offset per block
    col = 0
    for (c2, t0, t1) in blocks:
        nt = t1 - t0
        blk_off.append(col)
        for c in range(N_CORES):
            flat = sidx[c, t0:t1].reshape(-1)            # i = t*128+p
            wrapped = flat.reshape(-1, 16).T             # [16, nt*8]
            idx16[c, :, col:col + nt * 8] = np.tile(wrapped, (8, 1))
        col += nt * 8

    # item indices (2048 queries) wrapped the same way
    qi16 = np.zeros((N_CORES, P, (NW * P) // 16), np.int16)
    for c in range(N_CORES):
        qic = np.searchsorted(used_i, qitem[c])          # [P, NW] compact idx
        flat = qic.T.reshape(-1).astype(np.int16)        # i = w*128+p
        qi16[c] = np.tile(flat.reshape(-1, 16).T, (8, 1))

    slotf = np.transpose(slot, (0, 2, 1)).copy()         # [cores, 128, T_total]

    # window -> ordered stream tiles (both chunks)
    win_tiles = []
    for w in range(NW):
        ts = []
        for c2 in range(n_chunks):
            ts.extend(range(int(seg_off[w, c2]),
                            int(seg_off[w, c2] + T_seg[w, c2])))
        win_tiles.append(ts)

    edge_rows = sum(len(edges_u[0][w]) for w in range(NW))
    max_nt = max(t1 - t0 for (_, t0, t1) in blocks)
    # shared (max-over-cores) real idx count per block: the gather's
    # num_idxs covers only these, skipping tile-rounding pad descriptors
    blk_n = []
    for (c2, t0, t1), w in zip(blocks, blocks_w):
        off_rows = (t0 - int(seg_off[w, c2])) * P
        cap = (t1 - t0) * P
        n_here = np.clip(nreal[:, w, c2] - off_rows, 0, cap)
        blk_n.append(int(n_here.max()))
    # windows own contiguous stream-tile ranges (window-major layout)
    for w in range(NW):
        ts = win_tiles[w]
        assert ts == list(range(ts[0], ts[0] + len(ts)))
    max_ntw = max(len(ts) for ts in win_tiles)
    return dict(T_total=T_total, n_chunks=n_chunks, U_max=U_max,
                blocks=blocks, blk_off=blk_off, win_tiles=win_tiles,
                idx16=idx16, slotf=slotf, civq=civq, qi16=qi16, qpos=qpos,
                qitem=qitem, used_u=used_u, used_i=used_i, B=B, max_nt=max_nt,
                max_ntw=max_ntw, blk_n=blk_n,
                stream_rows=T_total * P, core0_edge_rows=edge_rows)


def _gather64(gp, out_ap, in_ap, idxs_ap, num_idxs, queue_num=0):
    """dma_gather with elem_size=64 bf16 (128B payload, 256B row stride).
    bass.dma_gather asserts elem%256B (a transpose-path restriction); the
    non-transpose ucode handles 128B elems (HW-validated), so build the
    instruction directly."""
    assert in_ap.ap[0][0] == DP
    return gp.add_instruction(
        mybir.InstDMAGatherAnt(
            name=gp.bass.get_next_instruction_name(),
            ins=[*gp.lower_ap_dma(in_ap, for_custom_bir_dma=True),
                 gp.lower_ap(idxs_ap),
                 gp.lower_val_access(gp.to_reg(num_idxs))],
            outs=[gp.lower_ap(out_ap)],
            transpose=False, num_idxs=num_idxs, elem_size=D,
            stride_bytes_256=(DP * 2) // 256, gen_mode=0,
            single_packet=False, queue_num=queue_num,
            sbuf_tokens_per_rank=0, sbuf_free_dim_per_rank=0,
            sbuf_free_dim_pad_per_rank=0, sbuf_byte_offset=0))


def build_nc(pl, I_rows, repeat=1, n_queues=4, do_gather=True, do_compute=True,
             dummy_oh=False):
    T_total = pl["T_total"]
    U_max = pl["U_max"]
    n_chunks = pl["n_chunks"]
    blocks = pl["blocks"]
    blk_off = pl["blk_off"]
    win_tiles = pl["win_tiles"]
    max_nt = pl["max_nt"]
    U_pad = max(U_max, (n_chunks - 1) * CH + 1)

    nc = bacc.Bacc("TRN2", target_bir_lowering=False, debug=False,
                   num_devices=N_CORES, num_swdge_queues=n_queues)
    user_emb = nc.dram_tensor("user_emb", [U_pad, DP], BF16,
                              kind="ExternalInput")
    idx_d = nc.dram_tensor("idx16", [P, T_total * 8], I16,
                           kind="ExternalInput")
    slot_d = nc.dram_tensor("slotf", [P, T_total], BF16, kind="ExternalInput")
    civ_d = nc.dram_tensor("civq", [P, NW], F32, kind="ExternalInput")
    item_q_d = nc.dram_tensor("item_q", [P, NW * D], BF16,
                              kind="ExternalInput")
    w1_d = nc.dram_tensor("w1", [D, 8], BF16, kind="ExternalInput")
    b1_d = nc.dram_tensor("b1", [8, 1], F32, kind="ExternalInput")
    w2_d = nc.dram_tensor("w2", [8, 1], BF16, kind="ExternalInput")
    b2_d = nc.dram_tensor("b2", [P, 1], F32, kind="ExternalInput")
    iota_d = nc.dram_tensor("iota", [P, P], BF16, kind="ExternalInput")
    result = nc.dram_tensor("result", [P, NW], F32, kind="ExternalOutput")

    with TileContext(nc) as tc:
        with tc.tile_pool(name="const", bufs=1) as cpool, \
             tc.tile_pool(name="meta", bufs=1) as mpool, \
             tc.tile_pool(name="g", bufs=16) as gpool, \
             tc.tile_pool(name="oh", bufs=4) as ohpool, \
             tc.tile_pool(name="x", bufs=4) as xpool, \
             tc.tile_pool(name="ps", bufs=2, space="PSUM") as pspool, \
             tc.tile_pool(name="psq", bufs=2, space="PSUM") as psq:

            iota_sb = cpool.tile([P, P], BF16)
            nc.sync.dma_start(out=iota_sb[:], in_=iota_d[:])
            w1_sb = cpool.tile([D, 8], BF16)
            nc.sync.dma_start(out=w1_sb[:], in_=w1_d[:])
            b1_sb = cpool.tile([8, 1], F32)
            nc.sync.dma_start(out=b1_sb[:], in_=b1_d[:])
            w2_sb = cpool.tile([8, 1], BF16)
            nc.sync.dma_start(out=w2_sb[:], in_=w2_d[:])
            b2_sb = cpool.tile([P, 1], F32)
            nc.sync.dma_start(out=b2_sb[:], in_=b2_d[:])
            civ_sb = cpool.tile([P, NW], F32)
            nc.sync.dma_start(out=civ_sb[:], in_=civ_d[:])
            ident = cpool.tile([P, P], BF16)
            make_identity(nc, ident[:])
            slot_sb = mpool.tile([P, T_total], BF16)
            nc.sync.dma_start(out=slot_sb[:], in_=slot_d[:])
            res_sb = cpool.tile([P, NW], F32)

            nc.gpsimd.load_library(library_config.mlp)

            # item rows, host-ordered by (window, partition): plain HWDGE DMA
            item_sb = cpool.tile([P, NW * D], BF16)
            nc.sync.dma_start(out=item_sb[:], in_=item_q_d[:])

            oh_const = None
            if dummy_oh:
                oh_const = cpool.tile([P, pl["max_ntw"] * P], BF16)
                nc.vector.memset(oh_const[:], 0.0)

            for rep in range(repeat):
                # user rows: one dma_gather per (window, chunk) block.
                # Queue = emission index % 4 (strict rotation: the Tile
                # SWDGE sem-lane rotation must stay aligned with queues);
                # blocks are pre-ordered in plan() so this balances load.
                # Pad idxs are 0 (row 0 fetched; -1 slots zero it in the
                # one-hot) so every output row is written -- trailing-(-1)
                # trimming would break the per-core-shared num_idxs.
                tile_loc = {}               # stream tile -> (buf, local_t)
                for bi, (c2, t0, t1) in enumerate(blocks):
                    nt = t1 - t0
                    g = gpool.tile([P, max_nt * D], BF16, tag="g")
                    if do_gather:
                        idx_k = mpool.tile([P, max_nt * 8], I16,
                                           tag=f"idx{bi % 8}")
                        nc.sync.dma_start(
                            out=idx_k[:, :nt * 8],
                            in_=idx_d[:, blk_off[bi]:blk_off[bi] + nt * 8])
                        src = user_emb[c2 * CH:min((c2 + 1) * CH, U_pad), :D]
                        _gather64(nc.gpsimd,
                                  g[:, :nt * D].rearrange("p (t e) -> p t e", e=D),
                                  src, idx_k[:, :nt * 8], nt * P,
                                  queue_num=bi % n_queues)
                    for t in range(t0, t1):
                        tile_loc[t] = (g, t - t0)

                if not do_compute:
                    # keep a data dependency on the gathers so they can't be
                    # dead-scheduled: one matmul per gather buffer
                    seen = set()
                    ps = pspool.tile([P, D], F32, tag="ps")
                    bufs = []
                    for t, (g, lt) in tile_loc.items():
                        if id(g) not in seen:
                            seen.add(id(g))
                            bufs.append(g)
                    for j, g in enumerate(bufs):
                        nc.tensor.matmul(out=ps[:], lhsT=iota_sb[:],
                                         rhs=g[:, :D],
                                         start=(j == 0), stop=(j == len(bufs) - 1))
                    xm = xpool.tile([P, D], BF16, tag="xm")
                    nc.vector.tensor_scalar_mul(out=xm[:], in0=ps[:],
                                                scalar1=civ_sb[:, 0:1])
                    nc.vector.tensor_scalar_mul(out=res_sb[:, 0:1],
                                                in0=xm[:, 0:1],
                                                scalar1=civ_sb[:, 0:1])
                    nc.sync.dma_start(out=result[:], in_=res_sb[:])
                    continue

                for w in range(NW):
                    ts = win_tiles[w]
                    ntw = len(ts)
                    ps = pspool.tile([P, D], F32, tag="ps")
                    # all of this window's one-hot tiles in ONE DVE
                    # tensor_tensor (is_equal of iota bcast vs slot bcast).
                    # tensor_tensor runs in single-port DVE mode, so unlike
                    # tensor_scalar it never locks GpSimd out of the SBUF
                    # port pair and SWDGE desc-gen streams on undisturbed.
                    if dummy_oh:
                        # timing-only variant: skip the one-hot generation
                        oh = oh_const
                    else:
                        oh = ohpool.tile([P, pl["max_ntw"] * P], BF16,
                                         tag="oh")
                        nc.vector.tensor_tensor(
                            out=oh[:, :ntw * P].rearrange("p (t c) -> p t c",
                                                          c=P),
                            in0=iota_sb[:].rearrange("p (t c) -> p t c", t=1)
                            .broadcast_to([P, ntw, P]),
                            in1=slot_sb[:, ts[0]:ts[0] + ntw]
                            .rearrange("p (t c) -> p t c", c=1)
                            .broadcast_to([P, ntw, P]),
                            op=mybir.AluOpType.is_equal)
                    for j, st in enumerate(ts):
                        g, lt = tile_loc[st]
                        nc.tensor.matmul(out=ps[:], lhsT=oh[:, j * P:(j + 1) * P],
                                         rhs=g[:, lt * D:lt * D + D],
                                         start=(j == 0), stop=(j == len(ts) - 1))
                    xm = xpool.tile([P, D], BF16, tag="xm")
                    nc.vector.tensor_scalar_mul(out=xm[:], in0=ps[:],
                                                scalar1=civ_sb[:, w:w + 1])
                    x = xpool.tile([P, D], BF16, tag="x")
                    nc.vector.tensor_mul(out=x[:], in0=xm[:],
                                         in1=item_sb[:, w * D:(w + 1) * D])
                    xT_ps = psq.tile([D, P], BF16, tag="xT_ps")
                    nc.tensor.transpose(out=xT_ps[:], in_=x[:], identity=ident[:])
                    xT = xpool.tile([D, P], BF16, tag="xT")
                    nc.scalar.activation(out=xT[:], in_=xT_ps[:],
                                         func=mybir.ActivationFunctionType.Copy)
                    h_ps = psq.tile([8, P], F32, tag="h_ps")
                    nc.tensor.matmul(out=h_ps[:], lhsT=w1_sb[:], rhs=xT[:],
                                     start=True, stop=True)
                    h = xpool.tile([8, P], BF16, tag="h")
                    nc.scalar.activation(out=h[:], in_=h_ps[:],
                                         func=mybir.ActivationFunctionType.Relu,
                                         bias=b1_sb[:])
                    o_ps = psq.tile([P, 1], F32, tag="o_ps")
                    nc.tensor.matmul(out=o_ps[:], lhsT=h[:], rhs=w2_sb[:],
                                     start=True, stop=True)
                    nc.scalar.activation(out=res_sb[:, w:w + 1], in_=o_ps[:],
                                         func=mybir.ActivationFunctionType.Sigmoid,
                                         bias=b2_sb[:])
                nc.sync.dma_start(out=result[:], in_=res_sb[:])
    nc.compile()
    return nc


def make_in_maps(pl, user_emb, item_emb, w1, b1, w2, b2):
    iota = np.broadcast_to(np.arange(P, dtype=np.float32),
                           (P, P)).astype(BF16_NP)
    U_pad = max(pl["U_max"], (pl["n_chunks"] - 1) * CH + 1)
    maps = []
    for c in range(N_CORES):
        uu = pl["used_u"][c]
        user_pad = np.zeros((U_pad, DP), BF16_NP)
        user_pad[:len(uu), :D] = user_emb[uu].astype(BF16_NP)
        # item rows laid out [p, w*D:(w+1)*D] = row of query (w, p)
        item_q = item_emb[pl["qitem"][c]].astype(BF16_NP)   # [P, NW, D]
        maps.append({
            "user_emb": user_pad,
            "idx16": pl["idx16"][c], "slotf": pl["slotf"][c].astype(BF16_NP),
            "civq": pl["civq"][c],
            "item_q": item_q.reshape(P, NW * D),
            "w1": w1.astype(BF16_NP), "b1": b1.reshape(8, 1),
            "w2": w2.reshape(8, 1).astype(BF16_NP),
            "b2": np.full((P, 1), float(b2.ravel()[0]), np.float32),
            "iota": iota,
        })
    return maps


def assemble(pl, core_results):
    out = np.zeros((pl["B"], 1), np.float32)
    for c in range(N_CORES):
        res = np.asarray(core_results[c], np.float32).reshape(P, NW)
        out[pl["qpos"][c].ravel(), 0] = res.ravel()
    return out


def prep_inputs(inputs):
    user_emb = np.ascontiguousarray(np.asarray(inputs["user_emb"], np.float32))
    item_emb = np.ascontiguousarray(np.asarray(inputs["item_emb"], np.float32))
    w1 = np.asarray(inputs["w1"], np.float32)
    b1 = np.asarray(inputs["b1"], np.float32)
    w2 = np.asarray(inputs["w2"], np.float32)
    b2 = np.asarray(inputs["b2"], np.float32)
    mu = np.asarray(inputs["member_users"]).astype(np.int64)
    mg = np.asarray(inputs["member_groups"]).astype(np.int64)
    gi = np.asarray(inputs["group_inputs"]).astype(np.int64)
    ii = np.asarray(inputs["item_inputs"]).astype(np.int64)
    G = int(np.asarray(inputs["num_groups"]))
    return user_emb, item_emb, w1, b1, w2, b2, mu, mg, gi, ii, G


def build_all(inputs):
    user_emb, item_emb, w1, b1, w2, b2, mu, mg, gi, ii, G = prep_inputs(inputs)
    pl = plan(mu, mg, gi, ii, G)
    nc = build_nc(pl, len(pl["used_i"]))
    maps = make_in_maps(pl, user_emb, item_emb, w1, b1, w2, b2)
    return pl, nc, maps


def kernel(**inputs):
    from concourse.bass_utils import run_bass_kernel_spmd
    pl, nc, maps = build_all(inputs)
    res = run_bass_kernel_spmd(nc, maps, list(range(N_CORES)))
    core_results = [res.results[c]["result"] for c in range(N_CORES)]
    return assemble(pl, core_results)



# revision 60
# speedup vs baseline: 2.3400x; 2.3400x over previous
"""ConsRec segment-reduce kernel for Trainium2 (8 NeuronCores, SPMD).

Strategy (v6): 16384 queries hit ~15.1K unique groups (only ~8% reuse), so
group means are never materialized.  Queries are dealt into 128-query
windows (16 per core x 8 cores) balanced by member count; each window's
member edges are duplicated per query.  Per core the device:
  1. gathers all member user-embedding rows with ONE `dma_gather`
     (InstDMAGatherAnt) per window, spread round-robin over the 4 SWDGE
     queues.  Descriptor generation for a gather runs on the Q7 cpu pair
     selected by queue_num (~6-9 ns/row per pair, HW measured), so 4
     queues generate descriptors 4x in parallel -- the single-queue v3
     kernel's bottleneck (~590 us/core gather-only).  dma_gather uses
     int16 indices, so the per-core table is re-segmented by WINDOW
     GROUP: each group of 4 windows gets its own <=32768-row segment of
     the unique rows it touches (rows in several groups are duplicated,
     ~11% extra table bytes) -- one gather per window instead of two
     chunk gathers, halving per-gather fixed overhead (~1.4 us each
     on-queue, HW measured) and tile padding.  First 4 windows' blocks
     are halved for ramp-in (20 blocks = exact multiple of 4 queues).
     Every gather has its own idx buffer so idx DMAs never sit on the
     queue-pair critical path (HW A/B: ~10-20 us).
  2. builds each window's ~32 one-hot tiles in ONE DVE tensor_tensor
     (is_equal of an iota row broadcast over tiles vs slot values
     broadcast over lanes).  tensor_tensor stays in single-port DVE mode;
     tensor_scalar one-hots (v4) entered 2-port perf mode, which locks
     GpSimd out of the shared SBUF port pair and starves SWDGE
     descriptor generation (v4 measured 360 us vs 102 us after this
     change -- see trainium-docs/memories/01-sbuf.md "DVE blocks DMA").
  3. PE-matmuls one-hot^T @ rows into a per-window PSUM [128 queries, 64]
     accumulating each query's member-sum directly,
  4. scales by 1/count, multiplies by host-ordered item rows (plain DMA,
     no item gather), runs the 64->8->1 MLP + sigmoid per window.
No group-means round trip, no phase barrier, no second gather pass.
HW time ~70-100 us/core (K-slope measured in quiet windows; 452 us
baseline).  Gather descriptor generation is the critical path: removing
all one-hot work saves only ~16 us while halving queues costs ~+99 us.
"""
import sys
sys.path.insert(0, '/opt/trn_rl_repo')
import numpy as np

import concourse.bacc as bacc
import concourse.bass as bass
import concourse.mybir as mybir
from concourse.tile import TileContext
from concourse.masks import make_identity
from concourse import library_config

N_CORES = 8
D = 64          # embedding dim
DP = 128        # padded row length (256B in bf16, dma_gather elem_size)
P = 128         # partitions / tile size / queries per window
NW = 16         # query windows per core (128 queries each)
CH = 32768      # user-table chunk rows (int16 index limit)

F32 = mybir.dt.float32
I16 = mybir.dt.int16
BF16 = mybir.dt.bfloat16
BF16_NP = mybir.dt.np(BF16)


def plan(member_users, member_groups, group_inputs, item_inputs, num_groups,
         split_below=4):
    G = int(num_groups)
    B = group_inputs.shape[0]
    NB = N_CORES * NW
    QPB = B // NB
    assert QPB == P, f"expected {P} queries/bin, got {QPB}"

    if not np.all(member_groups[:-1] <= member_groups[1:]):
        order = np.argsort(member_groups, kind='stable')
        member_groups = member_groups[order]
        member_users = member_users[order]
    starts = np.searchsorted(member_groups, np.arange(G + 1))
    cnt = (starts[1:] - starts[:-1]).astype(np.int64)
    qcnt = cnt[group_inputs]

    # deal queries into bins, snake order by member count -> equal bin loads
    qorder = np.argsort(-qcnt, kind='stable')
    bin_of = np.empty(B, np.int64)
    ar = np.arange(NB)
    for r in range(QPB):
        sl = qorder[r * NB:(r + 1) * NB]
        bin_of[sl] = ar if (r % 2 == 0) else ar[::-1]
    binload = np.bincount(bin_of, weights=qcnt.astype(np.float64),
                          minlength=NB).astype(np.int64)
    bsort = np.argsort(-binload, kind='stable')
    core_bins = [[] for _ in range(N_CORES)]
    for r in range(NW):
        row = bsort[r * N_CORES:(r + 1) * N_CORES]
        if r % 2 == 1:
            row = row[::-1]
        for c in range(N_CORES):
            core_bins[c].append(row[c])
    for c in range(N_CORES):
        core_bins[c].sort(key=lambda b: -binload[b])
    bin_queries = [np.where(bin_of == b)[0] for b in range(NB)]

    # per (core, window): raw edge lists (original user ids) + query info
    edges_u = [[None] * NW for _ in range(N_CORES)]   # user ids per edge
    edges_s = [[None] * NW for _ in range(N_CORES)]   # query col per edge
    civq = np.zeros((N_CORES, P, NW), np.float32)
    qpos = np.zeros((N_CORES, P, NW), np.int64)
    qitem = np.zeros((N_CORES, P, NW), np.int64)
    for c in range(N_CORES):
        for w in range(NW):
            qs = bin_queries[core_bins[c][w]]
            gl = group_inputs[qs]
            L = cnt[gl]
            off = starts[gl]
            tot = int(L.sum())
            if tot > 0:
                csum = np.cumsum(L) - L
                pos = (np.arange(tot) - np.repeat(csum, L) + np.repeat(off, L))
                edges_u[c][w] = member_users[pos]
                edges_s[c][w] = np.repeat(np.arange(P), L).astype(np.float32)
            else:
                edges_u[c][w] = np.empty(0, np.int64)
                edges_s[c][w] = np.empty(0, np.float32)
            civq[c, :, w] = 1.0 / np.maximum(cnt[gl], 1).astype(np.float32)
            qpos[c, :, w] = qs
            qitem[c, :, w] = item_inputs[qs]

    # per-core user compaction by WINDOW GROUP: each group of WG=4 windows
    # gets its own table segment holding the unique rows those windows
    # touch (~17K < 32768, so int16-addressable), placed at a fixed sg*CH
    # base in the per-core table.  One gather per window (vs two chunk
    # gathers) halves the per-gather fixed overhead and the tile-rounding
    # padding; rows used by several groups are simply duplicated in the
    # per-core table (~11% extra table bytes, same descriptor count).
    WG = 4
    n_seg = NW // WG
    seg_rows = [[None] * n_seg for _ in range(N_CORES)]
    comp = [[None] * NW for _ in range(N_CORES)]
    for c in range(N_CORES):
        for sg in range(n_seg):
            ws = range(sg * WG, (sg + 1) * WG)
            allu = np.concatenate([edges_u[c][w] for w in ws])
            if len(allu) == 0:
                allu = np.zeros(1, np.int64)
            uu = np.unique(allu)
            assert len(uu) <= CH, f"segment {sg}: {len(uu)} rows > {CH}"
            seg_rows[c][sg] = uu
            for w in ws:
                comp[c][w] = np.searchsorted(uu, edges_u[c][w])
    used_i = np.unique(item_inputs)

    # per-window tile counts, shared across cores
    T_seg = np.zeros(NW, np.int64)
    for c in range(N_CORES):
        for w in range(NW):
            n = len(comp[c][w])
            T_seg[w] = max(T_seg[w], (n + P - 1) // P, 1)

    # stream layout: window-major; seg_off[w] = first stream-tile index
    seg_off = np.zeros(NW, np.int64)
    off = 0
    for w in range(NW):
        seg_off[w] = off
        off += T_seg[w]
    T_total = int(off)

    # gather blocks: one per window; the first `split_below` windows are
    # halved for a fast ramp-in.  With split_below=4 that gives 20 blocks,
    # an exact multiple of the 4 SWDGE queues, so the strict q=i%4
    # rotation (required by SWDGE sem-lane grouping) stays balanced.
    blocks = []                           # (sg, t0, t1)
    blocks_w = []                         # window of each block
    for w in range(NW):
        sg = w // WG
        nt = int(T_seg[w])
        t0 = int(seg_off[w])
        if w < split_below and nt > 1:
            h = nt // 2
            blocks.append((sg, t0, t0 + h))
            blocks.append((sg, t0 + h, t0 + nt))
            blocks_w += [w, w]
        else:
            blocks.append((sg, t0, t0 + nt))
            blocks_w.append(w)

    # per-core stream data: local int16 idx + slot per stream tile
    # (slot values 0..127 / -1 sentinel: exactly representable in bf16)
    sidx = np.zeros((N_CORES, T_total, P), np.int16)
    slot = np.full((N_CORES, T_total, P), -1.0, np.float32)
    for c in range(N_CORES):
        for w in range(NW):
            li = comp[c][w].astype(np.int16)
            sl = edges_s[c][w]
            ord_ = np.argsort(li, kind='stable')   # HBM locality for the
            li = li[ord_]                          # 16-row packed descs
            sl = sl[ord_]
            n = len(li)
            t0 = int(seg_off[w])
            nt = int(T_seg[w])
            buf_i = np.zeros(nt * P, np.int16)
            buf_s = np.full(nt * P, -1.0, np.float32)
            buf_i[:n] = li
            buf_s[:n] = sl
            sidx[c, t0:t0 + nt] = buf_i.reshape(nt, P)
            slot[c, t0:t0 + nt] = buf_s.reshape(nt, P)

    # wrapped int16 index stream per gather block: [128, T_total*8]
    idx16 = np.zeros((N_CORES, P, T_total * 8), np.int16)
    blk_off = []                          # idx16 col offset per block
    col = 0
    for (_sg, t0, t1) in blocks:
        nt = t1 - t0
        blk_off.append(col)
        for c in range(N_CORES):
            flat = sidx[c, t0:t1].reshape(-1)            # i = t*128+p
            wrapped = flat.reshape(-1, 16).T             # [16, nt*8]
            idx16[c, :, col:col + nt * 8] = np.tile(wrapped, (8, 1))
        col += nt * 8

    # item indices (2048 queries) wrapped the same way
    qi16 = np.zeros((N_CORES, P, (NW * P) // 16), np.int16)
    for c in range(N_CORES):
        qic = np.searchsorted(used_i, qitem[c])          # [P, NW] compact idx
        flat = qic.T.reshape(-1).astype(np.int16)        # i = w*128+p
        qi16[c] = np.tile(flat.reshape(-1, 16).T, (8, 1))

    slotf = np.transpose(slot, (0, 2, 1)).copy()         # [cores, 128, T_total]

    # window -> ordered stream tiles (both chunks)
    win_tiles = []
    for w in range(NW):
        win_tiles.append(list(range(int(seg_off[w]),
                                    int(seg_off[w] + T_seg[w]))))

    max_nt = max(t1 - t0 for (_, t0, t1) in blocks)
    max_ntw = max(len(ts) for ts in win_tiles)
    return dict(T_total=T_total, n_seg=n_seg,
                blocks=blocks, blk_off=blk_off, win_tiles=win_tiles,
                idx16=idx16, slotf=slotf, civq=civq, qpos=qpos,
                qitem=qitem, seg_rows=seg_rows, used_i=used_i, B=B,
                max_nt=max_nt, max_ntw=max_ntw)


def _gather64(gp, out_ap, in_ap, idxs_ap, num_idxs, queue_num=0,
              single_packet=False):
    """dma_gather with elem_size=64 bf16 (128B payload, 256B row stride).
    bass.dma_gather asserts elem%256B (a transpose-path restriction); the
    non-transpose ucode handles 128B elems (HW-validated), so build the
    instruction directly."""
    assert in_ap.ap[0][0] == DP
    return gp.add_instruction(
        mybir.InstDMAGatherAnt(
            name=gp.bass.get_next_instruction_name(),
            ins=[*gp.lower_ap_dma(in_ap, for_custom_bir_dma=True),
                 gp.lower_ap(idxs_ap),
                 gp.lower_val_access(gp.to_reg(num_idxs))],
            outs=[gp.lower_ap(out_ap)],
            transpose=False, num_idxs=num_idxs, elem_size=D,
            stride_bytes_256=(DP * 2) // 256, gen_mode=0,
            single_packet=single_packet, queue_num=queue_num,
            sbuf_tokens_per_rank=0, sbuf_free_dim_per_rank=0,
            sbuf_free_dim_pad_per_rank=0, sbuf_byte_offset=0))


def build_nc(pl, I_rows, repeat=1, n_queues=4, do_gather=True, do_compute=True,
             dummy_oh=False, idx_bufs=0, single_packet=False,
             gather_bufs=16, oh_bufs=4):
    T_total = pl["T_total"]
    blocks = pl["blocks"]
    blk_off = pl["blk_off"]
    win_tiles = pl["win_tiles"]
    max_nt = pl["max_nt"]
    U_pad = pl["n_seg"] * CH

    nc = bacc.Bacc("TRN2", target_bir_lowering=False, debug=False,
                   num_devices=N_CORES, num_swdge_queues=n_queues)
    user_emb = nc.dram_tensor("user_emb", [U_pad, DP], BF16,
                              kind="ExternalInput")
    idx_d = nc.dram_tensor("idx16", [P, T_total * 8], I16,
                           kind="ExternalInput")
    slot_d = nc.dram_tensor("slotf", [P, T_total], BF16, kind="ExternalInput")
    civ_d = nc.dram_tensor("civq", [P, NW], F32, kind="ExternalInput")
    item_q_d = nc.dram_tensor("item_q", [P, NW * D], BF16,
                              kind="ExternalInput")
    w1_d = nc.dram_tensor("w1", [D, 8], BF16, kind="ExternalInput")
    b1_d = nc.dram_tensor("b1", [8, 1], F32, kind="ExternalInput")
    w2_d = nc.dram_tensor("w2", [8, 1], BF16, kind="ExternalInput")
    b2_d = nc.dram_tensor("b2", [P, 1], F32, kind="ExternalInput")
    iota_d = nc.dram_tensor("iota", [P, P], BF16, kind="ExternalInput")
    result = nc.dram_tensor("result", [P, NW], F32, kind="ExternalOutput")

    with TileContext(nc) as tc:
        with tc.tile_pool(name="const", bufs=1) as cpool, \
             tc.tile_pool(name="meta", bufs=1) as mpool, \
             tc.tile_pool(name="g", bufs=gather_bufs) as gpool, \
             tc.tile_pool(name="oh", bufs=oh_bufs) as ohpool, \
             tc.tile_pool(name="x", bufs=4) as xpool, \
             tc.tile_pool(name="ps", bufs=2, space="PSUM") as pspool, \
             tc.tile_pool(name="psq", bufs=2, space="PSUM") as psq:

            iota_sb = cpool.tile([P, P], BF16)
            nc.sync.dma_start(out=iota_sb[:], in_=iota_d[:])
            w1_sb = cpool.tile([D, 8], BF16)
            nc.sync.dma_start(out=w1_sb[:], in_=w1_d[:])
            b1_sb = cpool.tile([8, 1], F32)
            nc.sync.dma_start(out=b1_sb[:], in_=b1_d[:])
            w2_sb = cpool.tile([8, 1], BF16)
            nc.sync.dma_start(out=w2_sb[:], in_=w2_d[:])
            b2_sb = cpool.tile([P, 1], F32)
            nc.sync.dma_start(out=b2_sb[:], in_=b2_d[:])
            civ_sb = cpool.tile([P, NW], F32)
            nc.sync.dma_start(out=civ_sb[:], in_=civ_d[:])
            ident = cpool.tile([P, P], BF16)
            make_identity(nc, ident[:])
            slot_sb = mpool.tile([P, T_total], BF16)
            nc.sync.dma_start(out=slot_sb[:], in_=slot_d[:])
            res_sb = cpool.tile([P, NW], F32)

            nc.gpsimd.load_library(library_config.mlp)

            # item rows, host-ordered by (window, partition): plain HWDGE DMA
            item_sb = cpool.tile([P, NW * D], BF16)
            nc.sync.dma_start(out=item_sb[:], in_=item_q_d[:])

            oh_const = None
            if dummy_oh:
                oh_const = cpool.tile([P, pl["max_ntw"] * P], BF16)
                nc.vector.memset(oh_const[:], 0.0)

            for rep in range(repeat):
                # user rows: one dma_gather per (window, chunk) block.
                # Queue = emission index % 4 (strict rotation: the Tile
                # SWDGE sem-lane rotation must stay aligned with queues);
                # blocks are pre-ordered in plan() so this balances load.
                # Pad idxs are 0 (row 0 fetched; -1 slots zero it in the
                # one-hot) so every output row is written -- trailing-(-1)
                # trimming would break the per-core-shared num_idxs.
                tile_loc = {}               # stream tile -> (buf, local_t)
                # one idx buffer per block: all idx DMAs issue upfront, so
                # a queue's Q7 pair never waits on an idx load between
                # consecutive gathers (HW A/B: ~10-20us better than depth 8)
                nib = idx_bufs if idx_bufs > 0 else len(blocks)
                for bi, (sg, t0, t1) in enumerate(blocks):
                    nt = t1 - t0
                    g = gpool.tile([P, max_nt * D], BF16, tag="g")
                    if do_gather:
                        idx_k = mpool.tile([P, max_nt * 8], I16,
                                           tag=f"idx{bi % nib}")
                        nc.sync.dma_start(
                            out=idx_k[:, :nt * 8],
                            in_=idx_d[:, blk_off[bi]:blk_off[bi] + nt * 8])
                        src = user_emb[sg * CH:(sg + 1) * CH, :D]
                        _gather64(nc.gpsimd,
                                  g[:, :nt * D].rearrange("p (t e) -> p t e", e=D),
                                  src, idx_k[:, :nt * 8], nt * P,
                                  queue_num=bi % n_queues,
                                  single_packet=single_packet)
                    for t in range(t0, t1):
                        tile_loc[t] = (g, t - t0)

                if not do_compute:
                    # keep a data dependency on the gathers so they can't be
                    # dead-scheduled: one matmul per gather buffer
                    seen = set()
                    ps = pspool.tile([P, D], F32, tag="ps")
                    bufs = []
                    for t, (g, lt) in tile_loc.items():
                        if id(g) not in seen:
                            seen.add(id(g))
                            bufs.append(g)
                    for j, g in enumerate(bufs):
                        nc.tensor.matmul(out=ps[:], lhsT=iota_sb[:],
                                         rhs=g[:, :D],
                                         start=(j == 0), stop=(j == len(bufs) - 1))
                    xm = xpool.tile([P, D], BF16, tag="xm")
                    nc.vector.tensor_scalar_mul(out=xm[:], in0=ps[:],
                                                scalar1=civ_sb[:, 0:1])
                    nc.vector.tensor_scalar_mul(out=res_sb[:, 0:1],
                                                in0=xm[:, 0:1],
                                                scalar1=civ_sb[:, 0:1])
                    nc.sync.dma_start(out=result[:], in_=res_sb[:])
                    continue

                for w in range(NW):
                    ts = win_tiles[w]
                    ntw = len(ts)
                    ps = pspool.tile([P, D], F32, tag="ps")
                    # all of this window's one-hot tiles in ONE DVE
                    # tensor_tensor (is_equal of iota bcast vs slot bcast).
                    # tensor_tensor runs in single-port DVE mode, so unlike
                    # tensor_scalar it never locks GpSimd out of the SBUF
                    # port pair and SWDGE desc-gen streams on undisturbed.
                    if dummy_oh:
                        # timing-only variant: skip the one-hot generation
                        oh = oh_const
                    else:
                        oh = ohpool.tile([P, pl["max_ntw"] * P], BF16,
                                         tag="oh")
                        nc.vector.tensor_tensor(
                            out=oh[:, :ntw * P].rearrange("p (t c) -> p t c",
                                                          c=P),
                            in0=iota_sb[:].rearrange("p (t c) -> p t c", t=1)
                            .broadcast_to([P, ntw, P]),
                            in1=slot_sb[:, ts[0]:ts[0] + ntw]
                            .rearrange("p (t c) -> p t c", c=1)
                            .broadcast_to([P, ntw, P]),
                            op=mybir.AluOpType.is_equal)
                    for j, st in enumerate(ts):
                        g, lt = tile_loc[st]
                        nc.tensor.matmul(out=ps[:], lhsT=oh[:, j * P:(j + 1) * P],
                                         rhs=g[:, lt * D:lt * D + D],
                                         start=(j == 0), stop=(j == len(ts) - 1))
                    xm = xpool.tile([P, D], BF16, tag="xm")
                    nc.vector.tensor_scalar_mul(out=xm[:], in0=ps[:],
                                                scalar1=civ_sb[:, w:w + 1])
                    x = xpool.tile([P, D], BF16, tag="x")
                    nc.vector.tensor_mul(out=x[:], in0=xm[:],
                                         in1=item_sb[:, w * D:(w + 1) * D])
                    xT_ps = psq.tile([D, P], BF16, tag="xT_ps")
                    nc.tensor.transpose(out=xT_ps[:], in_=x[:], identity=ident[:])
                    xT = xpool.tile([D, P], BF16, tag="xT")
                    nc.scalar.activation(out=xT[:], in_=xT_ps[:],
                                         func=mybir.ActivationFunctionType.Copy)
                    h_ps = psq.tile([8, P], F32, tag="h_ps")
                    nc.tensor.matmul(out=h_ps[:], lhsT=w1_sb[:], rhs=xT[:],
                                     start=True, stop=True)
                    h = xpool.tile([8, P], BF16, tag="h")
                    nc.scalar.activation(out=h[:], in_=h_ps[:],
                                         func=mybir.ActivationFunctionType.Relu,
                                         bias=b1_sb[:])
                    o_ps = psq.tile([P, 1], F32, tag="o_ps")
                    nc.tensor.matmul(out=o_ps[:], lhsT=h[:], rhs=w2_sb[:],
                                     start=True, stop=True)
                    nc.scalar.activation(out=res_sb[:, w:w + 1], in_=o_ps[:],
                                         func=mybir.ActivationFunctionType.Sigmoid,
                                         bias=b2_sb[:])
                nc.sync.dma_start(out=result[:], in_=res_sb[:])
    nc.compile()
    return nc


def make_in_maps(pl, user_emb, item_emb, w1, b1, w2, b2):
    iota = np.broadcast_to(np.arange(P, dtype=np.float32),
                           (P, P)).astype(BF16_NP)
    U_pad = pl["n_seg"] * CH
    maps = []
    for c in range(N_CORES):
        user_pad = np.zeros((U_pad, DP), BF16_NP)
        for sg, uu in enumerate(pl["seg_rows"][c]):
            user_pad[sg * CH:sg * CH + len(uu), :D] = \
                user_emb[uu].astype(BF16_NP)
        # item rows laid out [p, w*D:(w+1)*D] = row of query (w, p)
        item_q = item_emb[pl["qitem"][c]].astype(BF16_NP)   # [P, NW, D]
        maps.append({
            "user_emb": user_pad,
            "idx16": pl["idx16"][c], "slotf": pl["slotf"][c].astype(BF16_NP),
            "civq": pl["civq"][c],
            "item_q": item_q.reshape(P, NW * D),
            "w1": w1.astype(BF16_NP), "b1": b1.reshape(8, 1),
            "w2": w2.reshape(8, 1).astype(BF16_NP),
            "b2": np.full((P, 1), float(b2.ravel()[0]), np.float32),
            "iota": iota,
        })
    return maps


def assemble(pl, core_results):
    out = np.zeros((pl["B"], 1), np.float32)
    for c in range(N_CORES):
        res = np.asarray(core_results[c], np.float32).reshape(P, NW)
        out[pl["qpos"][c].ravel(), 0] = res.ravel()
    return out


def prep_inputs(inputs):
    user_emb = np.ascontiguousarray(np.asarray(inputs["user_emb"], np.float32))
    item_emb = np.ascontiguousarray(np.asarray(inputs["item_emb"], np.float32))
    w1 = np.asarray(inputs["w1"], np.float32)
    b1 = np.asarray(inputs["b1"], np.float32)
    w2 = np.asarray(inputs["w2"], np.float32)
    b2 = np.asarray(inputs["b2"], np.float32)
    mu = np.asarray(inputs["member_users"]).astype(np.int64)
    mg = np.asarray(inputs["member_groups"]).astype(np.int64)
    gi = np.asarray(inputs["group_inputs"]).astype(np.int64)
    ii = np.asarray(inputs["item_inputs"]).astype(np.int64)
    G = int(np.asarray(inputs["num_groups"]))
    return user_emb, item_emb, w1, b1, w2, b2, mu, mg, gi, ii, G


def build_all(inputs):
    user_emb, item_emb, w1, b1, w2, b2, mu, mg, gi, ii, G = prep_inputs(inputs)
    pl = plan(mu, mg, gi, ii, G)
    nc = build_nc(pl, len(pl["used_i"]))
    maps = make_in_maps(pl, user_emb, item_emb, w1, b1, w2, b2)
    return pl, nc, maps


def kernel(**inputs):
    from concourse.bass_utils import run_bass_kernel_spmd
    pl, nc, maps = build_all(inputs)
    res = run_bass_kernel_spmd(nc, maps, list(range(N_CORES)))
    core_results = [res.results[c]["result"] for c in range(N_CORES)]
    return assemble(pl, core_results)



# revision 63
# speedup vs baseline: 2.8798x; 1.2307x over previous
"""ConsRec segment-reduce kernel for Trainium2 (8 NeuronCores, SPMD).

Strategy (v6): 16384 queries hit ~15.1K unique groups (only ~8% reuse), so
group means are never materialized.  Queries are dealt into 128-query
windows (16 per core x 8 cores) balanced by member count; each window's
member edges are duplicated per query.  Per core the device:
  1. gathers all member user-embedding rows with ONE `dma_gather`
     (InstDMAGatherAnt) per window, spread round-robin over the 4 SWDGE
     queues.  Descriptor generation for a gather runs on the Q7 cpu pair
     selected by queue_num (~6-9 ns/row per pair, HW measured), so 4
     queues generate descriptors 4x in parallel -- the single-queue v3
     kernel's bottleneck (~590 us/core gather-only).  dma_gather uses
     int16 indices, so the per-core table is re-segmented by WINDOW
     GROUP: each group of 4 windows gets its own <=32768-row segment of
     the unique rows it touches (rows in several groups are duplicated,
     ~11% extra table bytes) -- one gather per window instead of two
     chunk gathers, halving per-gather fixed overhead (~1.4 us each
     on-queue, HW measured) and tile padding.  First 4 windows' blocks
     are halved for ramp-in (20 blocks = exact multiple of 4 queues).
     Every gather has its own idx buffer so idx DMAs never sit on the
     queue-pair critical path (HW A/B: ~10-20 us).
  2. builds each window's ~32 one-hot tiles in ONE DVE tensor_tensor
     (is_equal of an iota row broadcast over tiles vs slot values
     broadcast over lanes).  tensor_tensor stays in single-port DVE mode;
     tensor_scalar one-hots (v4) entered 2-port perf mode, which locks
     GpSimd out of the shared SBUF port pair and starves SWDGE
     descriptor generation (v4 measured 360 us vs 102 us after this
     change -- see trainium-docs/memories/01-sbuf.md "DVE blocks DMA").
  3. PE-matmuls one-hot^T @ rows into a per-window PSUM [128 queries, 64]
     accumulating each query's member-sum directly,
  4. scales by 1/count, multiplies by host-ordered item rows (plain DMA,
     no item gather), runs the 64->8->1 MLP + sigmoid per window.
No group-means round trip, no phase barrier, no second gather pass.
HW time ~70-100 us/core (K-slope measured in quiet windows; 452 us
baseline).  Gather descriptor generation is the critical path: removing
all one-hot work saves only ~16 us while halving queues costs ~+99 us.
"""
import sys
sys.path.insert(0, '/opt/trn_rl_repo')
import numpy as np

import concourse.bacc as bacc
import concourse.bass as bass
import concourse.mybir as mybir
from concourse.tile import TileContext
from concourse.masks import make_identity
from concourse import library_config

N_CORES = 8
D = 64          # embedding dim
DP = 128        # padded row length (256B in bf16, dma_gather elem_size)
P = 128         # partitions / tile size / queries per window
NW = 16         # query windows per core (128 queries each)
CH = 32768      # user-table chunk rows (int16 index limit)

F32 = mybir.dt.float32
I16 = mybir.dt.int16
BF16 = mybir.dt.bfloat16
BF16_NP = mybir.dt.np(BF16)


def plan(member_users, member_groups, group_inputs, item_inputs, num_groups,
         split_below=4):
    G = int(num_groups)
    B = group_inputs.shape[0]
    NB = N_CORES * NW
    QPB = B // NB
    assert QPB == P, f"expected {P} queries/bin, got {QPB}"

    if not np.all(member_groups[:-1] <= member_groups[1:]):
        order = np.argsort(member_groups, kind='stable')
        member_groups = member_groups[order]
        member_users = member_users[order]
    starts = np.searchsorted(member_groups, np.arange(G + 1))
    cnt = (starts[1:] - starts[:-1]).astype(np.int64)
    qcnt = cnt[group_inputs]

    # deal queries into bins, snake order by member count -> equal bin loads
    qorder = np.argsort(-qcnt, kind='stable')
    bin_of = np.empty(B, np.int64)
    ar = np.arange(NB)
    for r in range(QPB):
        sl = qorder[r * NB:(r + 1) * NB]
        bin_of[sl] = ar if (r % 2 == 0) else ar[::-1]
    binload = np.bincount(bin_of, weights=qcnt.astype(np.float64),
                          minlength=NB).astype(np.int64)
    bsort = np.argsort(-binload, kind='stable')
    core_bins = [[] for _ in range(N_CORES)]
    for r in range(NW):
        row = bsort[r * N_CORES:(r + 1) * N_CORES]
        if r % 2 == 1:
            row = row[::-1]
        for c in range(N_CORES):
            core_bins[c].append(row[c])
    for c in range(N_CORES):
        core_bins[c].sort(key=lambda b: -binload[b])
    bin_queries = [np.where(bin_of == b)[0] for b in range(NB)]

    # per (core, window): raw edge lists (original user ids) + query info
    edges_u = [[None] * NW for _ in range(N_CORES)]   # user ids per edge
    edges_s = [[None] * NW for _ in range(N_CORES)]   # query col per edge
    civq = np.zeros((N_CORES, P, NW), np.float32)
    qpos = np.zeros((N_CORES, P, NW), np.int64)
    qitem = np.zeros((N_CORES, P, NW), np.int64)
    for c in range(N_CORES):
        for w in range(NW):
            qs = bin_queries[core_bins[c][w]]
            gl = group_inputs[qs]
            L = cnt[gl]
            off = starts[gl]
            tot = int(L.sum())
            if tot > 0:
                csum = np.cumsum(L) - L
                pos = (np.arange(tot) - np.repeat(csum, L) + np.repeat(off, L))
                edges_u[c][w] = member_users[pos]
                edges_s[c][w] = np.repeat(np.arange(P), L).astype(np.float32)
            else:
                edges_u[c][w] = np.empty(0, np.int64)
                edges_s[c][w] = np.empty(0, np.float32)
            civq[c, :, w] = 1.0 / np.maximum(cnt[gl], 1).astype(np.float32)
            qpos[c, :, w] = qs
            qitem[c, :, w] = item_inputs[qs]

    # per-core user compaction by WINDOW GROUP: each group of WG=4 windows
    # gets its own table segment holding the unique rows those windows
    # touch (~17K < 32768, so int16-addressable), placed at a fixed sg*CH
    # base in the per-core table.  One gather per window (vs two chunk
    # gathers) halves the per-gather fixed overhead and the tile-rounding
    # padding; rows used by several groups are simply duplicated in the
    # per-core table (~11% extra table bytes, same descriptor count).
    WG = 4
    n_seg = NW // WG
    seg_rows = [[None] * n_seg for _ in range(N_CORES)]
    comp = [[None] * NW for _ in range(N_CORES)]
    for c in range(N_CORES):
        for sg in range(n_seg):
            ws = range(sg * WG, (sg + 1) * WG)
            allu = np.concatenate([edges_u[c][w] for w in ws])
            if len(allu) == 0:
                allu = np.zeros(1, np.int64)
            uu = np.unique(allu)
            assert len(uu) <= CH, f"segment {sg}: {len(uu)} rows > {CH}"
            seg_rows[c][sg] = uu
            for w in ws:
                comp[c][w] = np.searchsorted(uu, edges_u[c][w])
    used_i = np.unique(item_inputs)

    # per-window tile counts, shared across cores
    T_seg = np.zeros(NW, np.int64)
    for c in range(N_CORES):
        for w in range(NW):
            n = len(comp[c][w])
            T_seg[w] = max(T_seg[w], (n + P - 1) // P, 1)

    # stream layout: window-major; seg_off[w] = first stream-tile index
    seg_off = np.zeros(NW, np.int64)
    off = 0
    for w in range(NW):
        seg_off[w] = off
        off += T_seg[w]
    T_total = int(off)

    # gather blocks: one per window; the first two windows are split in
    # quarters and the next two in halves for a fast ramp-in.  That gives
    # 24 blocks, an exact multiple of the 4 SWDGE queues, so the strict
    # q=i%4 rotation (required by SWDGE sem-lane grouping) stays balanced.
    blocks = []                           # (sg, t0, t1)
    blocks_w = []                         # window of each block
    for w in range(NW):
        sg = w // WG
        nt = int(T_seg[w])
        t0 = int(seg_off[w])
        if w < 2 and nt > 3:
            pieces = 4
        elif w < split_below and nt > 1:
            pieces = 2
        else:
            pieces = 1
        cuts = [t0 + (nt * i) // pieces for i in range(pieces + 1)]
        for a, b in zip(cuts[:-1], cuts[1:]):
            blocks.append((sg, a, b))
            blocks_w.append(w)

    # per-core stream data: local int16 idx + slot per stream tile
    # (slot values 0..127 / -1 sentinel: exactly representable in bf16)
    sidx = np.zeros((N_CORES, T_total, P), np.int16)
    slot = np.full((N_CORES, T_total, P), -1.0, np.float32)
    for c in range(N_CORES):
        for w in range(NW):
            li = comp[c][w].astype(np.int16)
            sl = edges_s[c][w]
            ord_ = np.argsort(li, kind='stable')   # HBM locality for the
            li = li[ord_]                          # 16-row packed descs
            sl = sl[ord_]
            n = len(li)
            t0 = int(seg_off[w])
            nt = int(T_seg[w])
            buf_i = np.zeros(nt * P, np.int16)
            buf_s = np.full(nt * P, -1.0, np.float32)
            buf_i[:n] = li
            buf_s[:n] = sl
            sidx[c, t0:t0 + nt] = buf_i.reshape(nt, P)
            slot[c, t0:t0 + nt] = buf_s.reshape(nt, P)

    # wrapped int16 index stream per gather block: [128, T_total*8]
    idx16 = np.zeros((N_CORES, P, T_total * 8), np.int16)
    blk_off = []                          # idx16 col offset per block
    col = 0
    for (_sg, t0, t1) in blocks:
        nt = t1 - t0
        blk_off.append(col)
        for c in range(N_CORES):
            flat = sidx[c, t0:t1].reshape(-1)            # i = t*128+p
            wrapped = flat.reshape(-1, 16).T             # [16, nt*8]
            idx16[c, :, col:col + nt * 8] = np.tile(wrapped, (8, 1))
        col += nt * 8

    # item indices (2048 queries) wrapped the same way
    qi16 = np.zeros((N_CORES, P, (NW * P) // 16), np.int16)
    for c in range(N_CORES):
        qic = np.searchsorted(used_i, qitem[c])          # [P, NW] compact idx
        flat = qic.T.reshape(-1).astype(np.int16)        # i = w*128+p
        qi16[c] = np.tile(flat.reshape(-1, 16).T, (8, 1))

    slotf = np.transpose(slot, (0, 2, 1)).copy()         # [cores, 128, T_total]

    # window -> ordered stream tiles (both chunks)
    win_tiles = []
    for w in range(NW):
        win_tiles.append(list(range(int(seg_off[w]),
                                    int(seg_off[w] + T_seg[w]))))

    max_nt = max(t1 - t0 for (_, t0, t1) in blocks)
    max_ntw = max(len(ts) for ts in win_tiles)
    return dict(T_total=T_total, n_seg=n_seg,
                blocks=blocks, blk_off=blk_off, win_tiles=win_tiles,
                idx16=idx16, slotf=slotf, civq=civq, qpos=qpos,
                qitem=qitem, seg_rows=seg_rows, used_i=used_i, B=B,
                max_nt=max_nt, max_ntw=max_ntw)


def _gather64(gp, out_ap, in_ap, idxs_ap, num_idxs, queue_num=0,
              single_packet=False):
    """dma_gather with elem_size=64 bf16 (128B payload, 256B row stride).
    bass.dma_gather asserts elem%256B (a transpose-path restriction); the
    non-transpose ucode handles 128B elems (HW-validated), so build the
    instruction directly."""
    assert in_ap.ap[0][0] == DP
    return gp.add_instruction(
        mybir.InstDMAGatherAnt(
            name=gp.bass.get_next_instruction_name(),
            ins=[*gp.lower_ap_dma(in_ap, for_custom_bir_dma=True),
                 gp.lower_ap(idxs_ap),
                 gp.lower_val_access(gp.to_reg(num_idxs))],
            outs=[gp.lower_ap(out_ap)],
            transpose=False, num_idxs=num_idxs, elem_size=D,
            stride_bytes_256=(DP * 2) // 256, gen_mode=0,
            single_packet=single_packet, queue_num=queue_num,
            sbuf_tokens_per_rank=0, sbuf_free_dim_per_rank=0,
            sbuf_free_dim_pad_per_rank=0, sbuf_byte_offset=0))


def build_nc(pl, I_rows, repeat=1, n_queues=4, do_gather=True, do_compute=True,
             dummy_oh=False, idx_bufs=0, single_packet=False,
             gather_bufs=16, oh_bufs=4):
    T_total = pl["T_total"]
    blocks = pl["blocks"]
    blk_off = pl["blk_off"]
    win_tiles = pl["win_tiles"]
    max_nt = pl["max_nt"]
    U_pad = pl["n_seg"] * CH

    nc = bacc.Bacc("TRN2", target_bir_lowering=False, debug=False,
                   num_devices=N_CORES, num_swdge_queues=n_queues)
    user_emb = nc.dram_tensor("user_emb", [U_pad, DP], BF16,
                              kind="ExternalInput")
    idx_d = nc.dram_tensor("idx16", [P, T_total * 8], I16,
                           kind="ExternalInput")
    slot_d = nc.dram_tensor("slotf", [P, T_total], BF16, kind="ExternalInput")
    civ_d = nc.dram_tensor("civq", [P, NW], F32, kind="ExternalInput")
    item_q_d = nc.dram_tensor("item_q", [P, NW * D], BF16,
                              kind="ExternalInput")
    w1_d = nc.dram_tensor("w1", [D, 8], BF16, kind="ExternalInput")
    b1_d = nc.dram_tensor("b1", [8, 1], F32, kind="ExternalInput")
    w2_d = nc.dram_tensor("w2", [8, 1], BF16, kind="ExternalInput")
    b2_d = nc.dram_tensor("b2", [P, 1], F32, kind="ExternalInput")
    iota_d = nc.dram_tensor("iota", [P, P], BF16, kind="ExternalInput")
    result = nc.dram_tensor("result", [P, NW], F32, kind="ExternalOutput")

    with TileContext(nc) as tc:
        with tc.tile_pool(name="const", bufs=1) as cpool, \
             tc.tile_pool(name="meta", bufs=1) as mpool, \
             tc.tile_pool(name="g", bufs=gather_bufs) as gpool, \
             tc.tile_pool(name="oh", bufs=oh_bufs) as ohpool, \
             tc.tile_pool(name="x", bufs=4) as xpool, \
             tc.tile_pool(name="ps", bufs=2, space="PSUM") as pspool, \
             tc.tile_pool(name="psq", bufs=2, space="PSUM") as psq:

            iota_sb = cpool.tile([P, P], BF16)
            nc.sync.dma_start(out=iota_sb[:], in_=iota_d[:])
            w1_sb = cpool.tile([D, 8], BF16)
            nc.sync.dma_start(out=w1_sb[:], in_=w1_d[:])
            b1_sb = cpool.tile([8, 1], F32)
            nc.sync.dma_start(out=b1_sb[:], in_=b1_d[:])
            w2_sb = cpool.tile([8, 1], BF16)
            nc.sync.dma_start(out=w2_sb[:], in_=w2_d[:])
            b2_sb = cpool.tile([P, 1], F32)
            nc.sync.dma_start(out=b2_sb[:], in_=b2_d[:])
            civ_sb = cpool.tile([P, NW], F32)
            nc.sync.dma_start(out=civ_sb[:], in_=civ_d[:])
            ident = cpool.tile([P, P], BF16)
            make_identity(nc, ident[:])
            slot_sb = mpool.tile([P, T_total], BF16)
            nc.sync.dma_start(out=slot_sb[:], in_=slot_d[:])
            res_sb = cpool.tile([P, NW], F32)

            nc.gpsimd.load_library(library_config.mlp)

            # item rows, host-ordered by (window, partition): plain HWDGE DMA
            item_sb = cpool.tile([P, NW * D], BF16)
            nc.sync.dma_start(out=item_sb[:], in_=item_q_d[:])

            oh_const = None
            if dummy_oh:
                oh_const = cpool.tile([P, pl["max_ntw"] * P], BF16)
                nc.vector.memset(oh_const[:], 0.0)

            for rep in range(repeat):
                # user rows: one dma_gather per (window, chunk) block.
                # Queue = emission index % 4 (strict rotation: the Tile
                # SWDGE sem-lane rotation must stay aligned with queues);
                # blocks are pre-ordered in plan() so this balances load.
                # Pad idxs are 0 (row 0 fetched; -1 slots zero it in the
                # one-hot) so every output row is written -- trailing-(-1)
                # trimming would break the per-core-shared num_idxs.
                tile_loc = {}               # stream tile -> (buf, local_t)
                # one idx buffer per block: all idx DMAs issue upfront, so
                # a queue's Q7 pair never waits on an idx load between
                # consecutive gathers (HW A/B: ~10-20us better than depth 8)
                nib = idx_bufs if idx_bufs > 0 else len(blocks)
                for bi, (sg, t0, t1) in enumerate(blocks):
                    nt = t1 - t0
                    g = gpool.tile([P, max_nt * D], BF16, tag="g")
                    if do_gather:
                        idx_k = mpool.tile([P, max_nt * 8], I16,
                                           tag=f"idx{bi % nib}")
                        nc.sync.dma_start(
                            out=idx_k[:, :nt * 8],
                            in_=idx_d[:, blk_off[bi]:blk_off[bi] + nt * 8])
                        src = user_emb[sg * CH:(sg + 1) * CH, :D]
                        _gather64(nc.gpsimd,
                                  g[:, :nt * D].rearrange("p (t e) -> p t e", e=D),
                                  src, idx_k[:, :nt * 8], nt * P,
                                  queue_num=bi % n_queues,
                                  single_packet=single_packet)
                    for t in range(t0, t1):
                        tile_loc[t] = (g, t - t0)

                if not do_compute:
                    # keep a data dependency on the gathers so they can't be
                    # dead-scheduled: one matmul per gather buffer
                    seen = set()
                    ps = pspool.tile([P, D], F32, tag="ps")
                    bufs = []
                    for t, (g, lt) in tile_loc.items():
                        if id(g) not in seen:
                            seen.add(id(g))
                            bufs.append(g)
                    for j, g in enumerate(bufs):
                        nc.tensor.matmul(out=ps[:], lhsT=iota_sb[:],
                                         rhs=g[:, :D],
                                         start=(j == 0), stop=(j == len(bufs) - 1))
                    xm = xpool.tile([P, D], BF16, tag="xm")
                    nc.vector.tensor_scalar_mul(out=xm[:], in0=ps[:],
                                                scalar1=civ_sb[:, 0:1])
                    nc.vector.tensor_scalar_mul(out=res_sb[:, 0:1],
                                                in0=xm[:, 0:1],
                                                scalar1=civ_sb[:, 0:1])
                    nc.sync.dma_start(out=result[:], in_=res_sb[:])
                    continue

                for w in range(NW):
                    ts = win_tiles[w]
                    ntw = len(ts)
                    ps = pspool.tile([P, D], F32, tag="ps")
                    # all of this window's one-hot tiles in ONE DVE
                    # tensor_tensor (is_equal of iota bcast vs slot bcast).
                    # tensor_tensor runs in single-port DVE mode, so unlike
                    # tensor_scalar it never locks GpSimd out of the SBUF
                    # port pair and SWDGE desc-gen streams on undisturbed.
                    if dummy_oh:
                        # timing-only variant: skip the one-hot generation
                        oh = oh_const
                    else:
                        oh = ohpool.tile([P, pl["max_ntw"] * P], BF16,
                                         tag="oh")
                        nc.vector.tensor_tensor(
                            out=oh[:, :ntw * P].rearrange("p (t c) -> p t c",
                                                          c=P),
                            in0=iota_sb[:].rearrange("p (t c) -> p t c", t=1)
                            .broadcast_to([P, ntw, P]),
                            in1=slot_sb[:, ts[0]:ts[0] + ntw]
                            .rearrange("p (t c) -> p t c", c=1)
                            .broadcast_to([P, ntw, P]),
                            op=mybir.AluOpType.is_equal)
                    for j, st in enumerate(ts):
                        g, lt = tile_loc[st]
                        nc.tensor.matmul(out=ps[:], lhsT=oh[:, j * P:(j + 1) * P],
                                         rhs=g[:, lt * D:lt * D + D],
                                         start=(j == 0), stop=(j == len(ts) - 1))
                    # item rows are pre-scaled by 1/count on the host, so
                    # the member-sum PSUM feeds the item multiply directly
                    x = xpool.tile([P, D], BF16, tag="x")
                    nc.vector.tensor_mul(out=x[:], in0=ps[:],
                                         in1=item_sb[:, w * D:(w + 1) * D])
                    xT_ps = psq.tile([D, P], BF16, tag="xT_ps")
                    nc.tensor.transpose(out=xT_ps[:], in_=x[:], identity=ident[:])
                    xT = xpool.tile([D, P], BF16, tag="xT")
                    nc.scalar.activation(out=xT[:], in_=xT_ps[:],
                                         func=mybir.ActivationFunctionType.Copy)
                    h_ps = psq.tile([8, P], F32, tag="h_ps")
                    nc.tensor.matmul(out=h_ps[:], lhsT=w1_sb[:], rhs=xT[:],
                                     start=True, stop=True)
                    h = xpool.tile([8, P], BF16, tag="h")
                    nc.scalar.activation(out=h[:], in_=h_ps[:],
                                         func=mybir.ActivationFunctionType.Relu,
                                         bias=b1_sb[:])
                    o_ps = psq.tile([P, 1], F32, tag="o_ps")
                    nc.tensor.matmul(out=o_ps[:], lhsT=h[:], rhs=w2_sb[:],
                                     start=True, stop=True)
                    nc.scalar.activation(out=res_sb[:, w:w + 1], in_=o_ps[:],
                                         func=mybir.ActivationFunctionType.Sigmoid,
                                         bias=b2_sb[:])
                nc.sync.dma_start(out=result[:], in_=res_sb[:])
    nc.compile()
    return nc


def make_in_maps(pl, user_emb, item_emb, w1, b1, w2, b2):
    iota = np.broadcast_to(np.arange(P, dtype=np.float32),
                           (P, P)).astype(BF16_NP)
    U_pad = pl["n_seg"] * CH
    maps = []
    for c in range(N_CORES):
        user_pad = np.zeros((U_pad, DP), BF16_NP)
        for sg, uu in enumerate(pl["seg_rows"][c]):
            user_pad[sg * CH:sg * CH + len(uu), :D] = \
                user_emb[uu].astype(BF16_NP)
        # item rows laid out [p, w*D:(w+1)*D] = row of query (w, p),
        # pre-scaled by that query's 1/member-count (folds the segment-mean
        # divide into the item multiply, dropping a DVE op per window)
        item_q = (item_emb[pl["qitem"][c]] *
                  pl["civq"][c][:, :, None]).astype(BF16_NP)  # [P, NW, D]
        maps.append({
            "user_emb": user_pad,
            "idx16": pl["idx16"][c], "slotf": pl["slotf"][c].astype(BF16_NP),
            "civq": pl["civq"][c],
            "item_q": item_q.reshape(P, NW * D),
            "w1": w1.astype(BF16_NP), "b1": b1.reshape(8, 1),
            "w2": w2.reshape(8, 1).astype(BF16_NP),
            "b2": np.full((P, 1), float(b2.ravel()[0]), np.float32),
            "iota": iota,
        })
    return maps


def assemble(pl, core_results):
    out = np.zeros((pl["B"], 1), np.float32)
    for c in range(N_CORES):
        res = np.asarray(core_results[c], np.float32).reshape(P, NW)
        out[pl["qpos"][c].ravel(), 0] = res.ravel()
    return out


def prep_inputs(inputs):
    user_emb = np.ascontiguousarray(np.asarray(inputs["user_emb"], np.float32))
    item_emb = np.ascontiguousarray(np.asarray(inputs["item_emb"], np.float32))
    w1 = np.asarray(inputs["w1"], np.float32)
    b1 = np.asarray(inputs["b1"], np.float32)
    w2 = np.asarray(inputs["w2"], np.float32)
    b2 = np.asarray(inputs["b2"], np.float32)
    mu = np.asarray(inputs["member_users"]).astype(np.int64)
    mg = np.asarray(inputs["member_groups"]).astype(np.int64)
    gi = np.asarray(inputs["group_inputs"]).astype(np.int64)
    ii = np.asarray(inputs["item_inputs"]).astype(np.int64)
    G = int(np.asarray(inputs["num_groups"]))
    return user_emb, item_emb, w1, b1, w2, b2, mu, mg, gi, ii, G


def build_all(inputs):
    user_emb, item_emb, w1, b1, w2, b2, mu, mg, gi, ii, G = prep_inputs(inputs)
    pl = plan(mu, mg, gi, ii, G)
    nc = build_nc(pl, len(pl["used_i"]))
    maps = make_in_maps(pl, user_emb, item_emb, w1, b1, w2, b2)
    return pl, nc, maps


def kernel(**inputs):
    from concourse.bass_utils import run_bass_kernel_spmd
    pl, nc, maps = build_all(inputs)
    res = run_bass_kernel_spmd(nc, maps, list(range(N_CORES)))
    core_results = [res.results[c]["result"] for c in range(N_CORES)]
    return assemble(pl, core_results)

